# revision 37
# baseline (speedup 1.0000x reference)
"""BatchTopK SAE kernel for 8 Trainium2 NeuronCores.

Strategy (tensor-parallel over d_sae for both matmuls):
  Launch 1 (encode): each core computes scores = relu(diff @ W_enc_slice
      + b_enc_slice) * dec_norms_slice for its F/8-feature slice, over the
      full batch, in bf16 matmul / f32 PSUM. Exports f32 scores.
  Host: exact global top-(k*B) selection over the device scores.
      Elements within +-DELTA of the device threshold are re-scored in f64
      ("ground truth"); the truth ordering fills the mask to exactly k*B.
      (The f64-truth mask coincides with the f32 jax reference mask: boundary
      score gaps ~1.6e-7 exceed f32 rounding noise.)
  Launch 2 (decode): each core computes a partial reconstruction
      partial = W_dec_slice.T @ sparse_acts_slice in bf16 / f32 PSUM.
  Host: sum the 8 partials, add b_dec.

All DRAM tensors use pre-tiled block layouts (built on host) so every DMA
transfer is a large contiguous region; strided narrow-row DMAs measured at
<170 GB/s while blocked ones run near line rate.

kernel() accepts FULL inputs and returns the FULL output.
"""

import os

import numpy as np
import ml_dtypes

import concourse.bass as bass  # noqa: F401
import concourse.mybir as mybir
import concourse.tile as tile
from concourse import bacc
from concourse.bass_utils import run_bass_kernel_spmd

BF16 = ml_dtypes.bfloat16
FP8 = ml_dtypes.float8_e4m3
N_CORES = 8
P = 128          # partitions
C = 512          # matmul free-dim chunk (one PSUM bank of f32)
DELTA = 2e-3     # f64 re-score band half-width (bf16 encode)
DELTA8 = 4.5e-2  # f64 re-score band half-width (fp8 encode)
WSCALE = 32.0    # fp8 weight pre-scale (keeps W_enc out of the e4m3 denormals)
USE_FP8 = bool(int(os.environ.get("KERNEL_FP8", "1")))

# Set by the harness to request tracing; timings land in LAST_EXEC_NS.
TRACE = bool(int(os.environ.get("KERNEL_TRACE", "0")))
LAST_EXEC_NS = []
LAST_PROFILE = []
LAST_TRACE = []

if TRACE:
    # The agent image's `antenv` lacks `axon_hooks`, so boot() skipped NTFF
    # hook registration. Recreate the module and register the ctypes hook so
    # run_bass_kernel_spmd(trace=True) can profile. Best effort only.
    try:
        import sys as _sys
        import types as _types

        try:
            from antenv import axon_hooks as _ah  # noqa: F401
        except ImportError:
            import antenv as _antenv

            _mod = _types.ModuleType("antenv.axon_hooks")
            _hook_box = [None]
            _mod.set_axon_ntff_profile_hook = (
                lambda h: _hook_box.__setitem__(0, h))
            _mod.get_axon_ntff_profile_hook = lambda: _hook_box[0]
            _sys.modules["antenv.axon_hooks"] = _mod
            _antenv.axon_hooks = _mod
            from trn_agent_boot.trn_boot import _ntff_profile_via_ctypes

            _mod.set_axon_ntff_profile_hook(
                _ntff_profile_via_ctypes("/opt/axon/libaxon_pjrt.so"))
        import concourse.bass_utils as _bu

        _bu.upload_artifacts = lambda tmpdir: tmpdir
    except Exception as _e:  # pragma: no cover
        print(f"kernel.py: NTFF trace hook setup failed: {_e}")

_BUILD_CACHE = {}


def _ln64(v):
    m = v.mean(axis=1, keepdims=True)
    var = ((v - m) ** 2).mean(axis=1, keepdims=True)
    return (v - m) / np.sqrt(var + 1e-8)


def _build_encode(D, FS, B):
    """Per-core encode: s = relu(psum * n + b*n) in one ACT op.

    DRAM (block layouts):
      dT  [NM, P, KT*C] bf16   (diff.T blocked by m-group)
      w   [KT, P, FS]   bf16   (W_enc slice blocked by k-tile)
      bn2 [FT, P] f32 (= b*n), nrm [FT, P] f32
      s   [NM, FT, P, C] f32 out
    """
    KT = D // P
    FT = FS // P
    NM = B // C

    nc = bacc.Bacc("TRN2", target_bir_lowering=False, debug=False,
                   num_devices=N_CORES)
    dT = nc.dram_tensor("dT", [NM, P, KT * C], mybir.dt.bfloat16,
                        kind="ExternalInput")
    w = nc.dram_tensor("w", [KT, P, FS], mybir.dt.bfloat16,
                       kind="ExternalInput")
    bn2 = nc.dram_tensor("bn2", [FT, P], mybir.dt.float32,
                         kind="ExternalInput")
    nrm = nc.dram_tensor("nrm", [FT, P], mybir.dt.float32,
                         kind="ExternalInput")
    s = nc.dram_tensor("s", [NM, FT, P, C], mybir.dt.float32,
                       kind="ExternalOutput")

    with tile.TileContext(nc) as tc:
        with (
            tc.tile_pool(name="resident", bufs=1) as res,
            tc.tile_pool(name="psum", bufs=4, space="PSUM") as psum_pool,
            tc.tile_pool(name="stage", bufs=8) as stage,
        ):
            bn_sb = res.tile([P, FT], mybir.dt.float32, name="bn_sb")
            nc.sync.dma_start(bn_sb[:], bn2.ap().rearrange("a p -> p a"))
            nrm_sb = res.tile([P, FT], mybir.dt.float32, name="nrm_sb")
            nc.sync.dma_start(nrm_sb[:], nrm.ap().rearrange("a p -> p a"))

            # Split every load into column sub-transfers so they spread
            # across DMA queues (single-queue rate is ~75 GB/s).
            w_sb = [res.tile([P, FS], mybir.dt.bfloat16, name=f"w_{ki}")
                    for ki in range(KT)]
            for ki in range(KT):
                h = FS // 2
                for q in range(2):
                    nc.sync.dma_start(w_sb[ki][:, q * h:(q + 1) * h],
                                      w.ap()[ki, :, q * h:(q + 1) * h])
            dT_sb = [res.tile([P, KT * C], mybir.dt.bfloat16, name=f"dT_{mg}")
                     for mg in range(NM)]
            for mg in range(NM):
                h = KT * C // 4
                for q in range(4):
                    nc.sync.dma_start(dT_sb[mg][:, q * h:(q + 1) * h],
                                      dT.ap()[mg, :, q * h:(q + 1) * h])

            for mg in range(NM):
                for fi in range(FT):
                    pt = psum_pool.tile([P, C], mybir.dt.float32,
                                        name="pe", tag="pe")
                    for ki in range(KT):
                        nc.tensor.matmul(
                            pt[:],
                            lhsT=w_sb[ki][:, fi * P:(fi + 1) * P],
                            rhs=dT_sb[mg][:, ki * C:(ki + 1) * C],
                            start=(ki == 0), stop=(ki == KT - 1),
                        )
                    out_t = stage.tile([P, C], mybir.dt.float32,
                                       name="score_t", tag="score")
                    nc.scalar.activation(
                        out_t[:], pt[:],
                        mybir.ActivationFunctionType.Relu,
                        bias=bn_sb[:, fi:fi + 1],
                        scale=nrm_sb[:, fi:fi + 1],
                    )
                    nc.sync.dma_start(s.ap()[mg, fi], out_t[:])
    nc.compile()
    return nc


def _build_encode_fp8(D, FS, B, zero_bias):
    """Per-core fp8 DoubleRow encode: s_bf16 = relu(psum * (n/WSCALE) + b*n).

    DRAM (block layouts):
      d8  [NM, P, KT*C]  fp8e4m3  (diff.T blocked by m-group)
      w8  [KP, P, 2*FS]  fp8e4m3  (W_enc*WSCALE, k-tile PAIRS for DoubleRow)
      bnn [FT, P] f32 (= b*n), nsc [FT, P] f32 (= n/WSCALE)
      s   [NM, FT, P, C] bf16 out

    With b == 0 the epilogue alternates ACT/DVE per f-tile: the PE emits a
    psum group every ~0.9us (fp8 DoubleRow) while one ACT pass costs ~0.7us,
    so a single engine would gate psum recycling.
    """
    KT = D // P
    KP = KT // 2
    FT = FS // P
    NM = B // C

    nc = bacc.Bacc("TRN2", target_bir_lowering=False, debug=False,
                   num_devices=N_CORES)
    d8 = nc.dram_tensor("d8", [NM, P, KT * C], mybir.dt.float8e4,
                        kind="ExternalInput")
    w8 = nc.dram_tensor("w8", [KP, P, 2 * FS], mybir.dt.float8e4,
                        kind="ExternalInput")
    bnn = nc.dram_tensor("bnn", [FT, P], mybir.dt.float32,
                         kind="ExternalInput")
    nsc = nc.dram_tensor("nsc", [FT, P], mybir.dt.float32,
                         kind="ExternalInput")
    s = nc.dram_tensor("s", [NM, FT, P, C], mybir.dt.bfloat16,
                       kind="ExternalOutput")

    with tile.TileContext(nc) as tc:
        with (
            tc.tile_pool(name="resident", bufs=1) as res,
            tc.tile_pool(name="psum", bufs=4, space="PSUM") as psum_pool,
            tc.tile_pool(name="stage", bufs=8) as stage,
        ):
            w_sb = [res.tile([P, 2, FS], mybir.dt.float8e4, name=f"w8_{kp}")
                    for kp in range(KP)]
            for kp in range(KP):
                nc.sync.dma_start(w_sb[kp][:],
                                  w8.ap()[kp].rearrange("p (t f) -> p t f", t=2))
            dT_sb = [res.tile([P, KT, C], mybir.dt.float8e4, name=f"d8_{mg}")
                     for mg in range(NM)]
            for mg in range(NM):
                h = KT // 2
                for q in range(2):
                    nc.sync.dma_start(
                        dT_sb[mg][:, q * h:(q + 1) * h, :],
                        d8.ap()[mg, :, q * h * C:(q + 1) * h * C]
                        .rearrange("p (a c) -> p a c", c=C))
            bn_sb = res.tile([P, FT], mybir.dt.float32, name="bn_sb")
            nc.sync.dma_start(bn_sb[:], bnn.ap().rearrange("a p -> p a"))
            ns_sb = res.tile([P, FT], mybir.dt.float32, name="ns_sb")
            nc.sync.dma_start(ns_sb[:], nsc.ap().rearrange("a p -> p a"))

            for mg in range(NM):
                for fi in range(FT):
                    pt = psum_pool.tile([P, C], mybir.dt.float32,
                                        name="pe", tag="pe")
                    for kp in range(KP):
                        nc.tensor.matmul(
                            pt[:],
                            lhsT=w_sb[kp][:, :, fi * P:(fi + 1) * P],
                            rhs=dT_sb[mg][:, 2 * kp:2 * kp + 2, :],
                            start=(kp == 0), stop=(kp == KP - 1),
                            perf_mode=mybir.MatmulPerfMode.DoubleRow,
                        )
                    out_t = stage.tile([P, C], mybir.dt.bfloat16,
                                       name="score_t", tag="score")
                    if zero_bias and fi % 2 == 1:
                        nc.vector.tensor_scalar(
                            out_t[:], pt[:], 0.0, ns_sb[:, fi:fi + 1],
                            op0=mybir.AluOpType.max,
                            op1=mybir.AluOpType.mult,
                        )
                    else:
                        nc.scalar.activation(
                            out_t[:], pt[:],
                            mybir.ActivationFunctionType.Relu,
                            bias=bn_sb[:, fi:fi + 1],
                            scale=ns_sb[:, fi:fi + 1],
                        )
                    nc.sync.dma_start(s.ap()[mg, fi], out_t[:])
    nc.compile()
    return nc


def _build_decode(D, FS, B):
    """Per-core decode partial: pr = W_dec_slice.T @ sa_slice.

    DRAM (block layouts):
      sa [NM, P, FT*C] bf16, wd [FT, P, D] bf16, pr [NM, DT, P, C] f32 out.
    """
    FT = FS // P
    DT_ = D // P
    NM = B // C

    nc = bacc.Bacc("TRN2", target_bir_lowering=False, debug=False,
                   num_devices=N_CORES)
    sa = nc.dram_tensor("sa", [NM, P, FT * C], mybir.dt.bfloat16,
                        kind="ExternalInput")
    wd = nc.dram_tensor("wd", [FT, P, D], mybir.dt.bfloat16,
                        kind="ExternalInput")
    pr = nc.dram_tensor("pr", [NM, DT_, P, C], mybir.dt.float32,
                        kind="ExternalOutput")

    with tile.TileContext(nc) as tc:
        with (
            tc.tile_pool(name="resident", bufs=1) as res,
            tc.tile_pool(name="psum", bufs=4, space="PSUM") as psum_pool,
            tc.tile_pool(name="stage", bufs=8) as stage,
        ):
            wd_sb = [res.tile([P, D], mybir.dt.bfloat16, name=f"wd_{fi}")
                     for fi in range(FT)]
            for fi in range(FT):
                nc.sync.dma_start(wd_sb[fi][:], wd.ap()[fi])
            sa_sb = [res.tile([P, FT * C], mybir.dt.bfloat16, name=f"sa_{mg}")
                     for mg in range(NM)]
            for mg in range(NM):
                h = FT * C // 4
                for q in range(4):
                    nc.sync.dma_start(sa_sb[mg][:, q * h:(q + 1) * h],
                                      sa.ap()[mg, :, q * h:(q + 1) * h])

            # Contraction (fi) outer within each half-group of 4 psum banks:
            # the first matmuls only need wd_f0 + sa_mg0. For mg 0 the fi
            # contraction of both halves is phase-split so the first ~14us
            # of PE work touches only wd_f0..7 (PSUM accumulation stays open
            # across the interleave).
            DH = min(4, DT_)
            NDH = DT_ // DH
            FHALF = FT // 2
            for mg in range(NM):
                pts_all = [[psum_pool.tile([P, C], mybir.dt.float32,
                                           name=f"pd{dh}_{j}",
                                           tag=f"pd{dh}_{j}", bufs=1)
                            for j in range(DH)] for dh in range(NDH)]
                if mg == 0 and FT % 2 == 0 and NDH > 1:
                    phases = [(dh, fr) for fr in (range(0, FHALF),
                                                  range(FHALF, FT))
                              for dh in range(NDH)]
                else:
                    phases = [(dh, range(FT)) for dh in range(NDH)]
                for dh, frange in phases:
                    for fi in frange:
                        for j in range(DH):
                            di = dh * DH + j
                            nc.tensor.matmul(
                                pts_all[dh][j][:],
                                lhsT=wd_sb[fi][:, di * P:(di + 1) * P],
                                rhs=sa_sb[mg][:, fi * C:(fi + 1) * C],
                                start=(fi == 0), stop=(fi == FT - 1),
                            )
                    if frange[-1] == FT - 1:
                        for j in range(DH):
                            di = dh * DH + j
                            out_t = stage.tile([P, C], mybir.dt.float32,
                                               name="prt_t", tag="prt")
                            nc.vector.tensor_copy(out_t[:], pts_all[dh][j][:])
                            nc.sync.dma_start(pr.ap()[mg, di], out_t[:])
    nc.compile()
    return nc


def _get_kernels(D, FS, B, fp8, zero_bias):
    key = (D, FS, B, fp8, zero_bias)
    if key not in _BUILD_CACHE:
        enc = (_build_encode_fp8(D, FS, B, zero_bias) if fp8
               else _build_encode(D, FS, B))
        _BUILD_CACHE[key] = (enc, _build_decode(D, FS, B))
    return _BUILD_CACHE[key]


def _chunked_preact64(diff64, W64T, b64, bb, ff, chunk=65536):
    """f64 pre-activations for element list (bb[i], ff[i])."""
    out = np.empty(bb.size, dtype=np.float64)
    for i in range(0, bb.size, chunk):
        sl = slice(i, min(i + chunk, bb.size))
        out[sl] = (np.einsum("ij,ij->i", diff64[bb[sl]], W64T[ff[sl]])
                   + b64[ff[sl]])
    return out


def _run(nc, in_maps):
    res = run_bass_kernel_spmd(nc, in_maps, list(range(N_CORES)), trace=TRACE)
    if TRACE:
        LAST_EXEC_NS.append(res.exec_time_ns)
        LAST_PROFILE.append(res.profile_json)
        if res.instructions_and_trace is not None:
            LAST_TRACE.append(res.instructions_and_trace[1])
    return res.results


def kernel(x, W_enc, b_enc, W_dec, b_dec, k):
    k = int(k)
    B = x.shape[0]
    D = W_enc.shape[0]
    F = W_enc.shape[1]
    FS = F // N_CORES
    KT, FT, NM = D // P, FS // P, B // C
    kB = k * B

    x = np.asarray(x, dtype=np.float32)
    W_enc = np.asarray(W_enc, dtype=np.float32)
    b_enc = np.asarray(b_enc, dtype=np.float32)
    W_dec = np.asarray(W_dec, dtype=np.float32)
    b_dec = np.asarray(b_dec, dtype=np.float32)

    zero_bias = not bool(np.any(b_enc))
    enc_nc, dec_nc = _get_kernels(D, FS, B, USE_FP8, zero_bias)

    # ---- host prep: f64 LN-diff chain and decoder norms ----
    x64 = x.astype(np.float64)
    diff64 = _ln64(_ln64(x64[:, D:]) - _ln64(x64[:, :D]))       # [B, D]
    n64 = np.sqrt((W_dec.astype(np.float64) ** 2).sum(axis=1))  # [F]
    nrm = n64.astype(np.float32)
    b64 = b_enc.astype(np.float64)

    in_maps = []
    if USE_FP8:
        KP = KT // 2
        diffT_8 = diff64.T.astype(np.float32).astype(FP8)
        d_blk = np.ascontiguousarray(
            diffT_8.reshape(KT, P, NM, C).transpose(2, 1, 0, 3)
            .reshape(NM, P, KT * C))
        for c in range(N_CORES):
            sl = slice(c * FS, (c + 1) * FS)
            w8_blk = np.ascontiguousarray(
                (W_enc[:, sl] * np.float32(WSCALE)).astype(FP8)
                .reshape(KP, 2, P, FS).transpose(0, 2, 1, 3)
                .reshape(KP, P, 2 * FS))
            in_maps.append({
                "d8": d_blk,
                "w8": w8_blk,
                "bnn": np.ascontiguousarray(
                    (b64[sl] * n64[sl]).astype(np.float32).reshape(FT, P)),
                "nsc": np.ascontiguousarray(
                    (n64[sl] / WSCALE).astype(np.float32).reshape(FT, P)),
            })
        delta = DELTA8
    else:
        diffT_bf = diff64.T.astype(BF16)
        dT_blk = np.ascontiguousarray(
            diffT_bf.reshape(KT, P, NM, C).transpose(2, 1, 0, 3)
            .reshape(NM, P, KT * C))
        for c in range(N_CORES):
            sl = slice(c * FS, (c + 1) * FS)
            w_blk = np.ascontiguousarray(
                W_enc[:, sl].astype(BF16).reshape(KT, P, FS))
            in_maps.append({
                "dT": dT_blk,
                "w": w_blk,
                "bn2": np.ascontiguousarray(
                    (b64[sl] * n64[sl]).astype(np.float32).reshape(FT, P)),
                "nrm": np.ascontiguousarray(nrm[sl].reshape(FT, P)),
            })
        delta = DELTA
    enc_out = _run(enc_nc, in_maps)
    # s blocks per core: [NM, FT, P, C]; element (c, mg, fi, p, j) is
    # feature f = c*FS + fi*P + p, batch b = mg*C + j.
    s_blk = np.stack([enc_out[c]["s"] for c in range(N_CORES)], axis=0)
    if s_blk.dtype != np.float32:
        s_blk = s_blk.astype(np.float32)

    # ---- host: exact top-(k*B) with f64 band repair ----
    flat = s_blk.reshape(-1)
    tau = np.partition(flat, flat.size - kB)[flat.size - kB]
    mask = flat >= tau + delta
    n_in = int(mask.sum())
    band = np.nonzero((flat > tau - delta) & (flat < tau + delta))[0]
    need = kB - n_in
    cc, mm, fifi, pp, jj = np.unravel_index(band, s_blk.shape)
    ff = cc * FS + fifi * P + pp
    bb = mm * C + jj
    W64T = np.ascontiguousarray(W_enc.astype(np.float64).T)     # [F, D]
    acts64_band = np.maximum(
        _chunked_preact64(diff64, W64T, b64, bb, ff), 0.0)
    s64_band = acts64_band * n64[ff]
    order = np.argsort(-s64_band, kind="stable")
    sel_band = order[:need]
    mask[band[sel_band]] = True

    # ---- sparse acts, masked, bf16 ----
    if USE_FP8:
        # fp8 scores are too noisy to recover acts from; rebuild every
        # selected activation from the f64 ground truth instead.
        sa_flat = np.zeros(flat.size, dtype=BF16)
        sa_flat[band[sel_band]] = acts64_band[sel_band].astype(BF16)
        ic = np.nonzero(mask & (flat >= tau + delta))[0]
        cc2, mm2, fifi2, pp2, jj2 = np.unravel_index(ic, s_blk.shape)
        ff2 = cc2 * FS + fifi2 * P + pp2
        bb2 = mm2 * C + jj2
        acts64_ic = np.maximum(
            _chunked_preact64(diff64, W64T, b64, bb2, ff2), 0.0)
        sa_flat[ic] = acts64_ic.astype(BF16)
        sa_all = sa_flat.reshape(s_blk.shape)
    else:
        recip = (np.float32(1.0) / nrm)                         # [F]
        acts = s_blk * recip.reshape(N_CORES, 1, FT, P, 1)
        acts *= mask.reshape(s_blk.shape)
        sa_all = acts.astype(BF16)                              # [8,NM,FT,P,C]

    in_maps2 = []
    for c in range(N_CORES):
        sl = slice(c * FS, (c + 1) * FS)
        sa_blk = np.ascontiguousarray(
            sa_all[c].transpose(0, 2, 1, 3).reshape(NM, P, FT * C))
        wd_blk = np.ascontiguousarray(
            W_dec[sl].astype(BF16).reshape(FT, P, D))
        in_maps2.append({"sa": sa_blk, "wd": wd_blk})
    dec_out = _run(dec_nc, in_maps2)

    acc = dec_out[0]["pr"].astype(np.float64)
    for c in range(1, N_CORES):
        acc += dec_out[c]["pr"]
    # [NM, DT, P, C] -> [B, D]
    reconT = acc.transpose(1, 2, 0, 3).reshape(D, B)
    recon = reconT.T.astype(np.float32) + b_dec[None, :]
    return recon.astype(np.float32)
